# revision 1
# baseline (speedup 1.0000x reference)
"""Trainium2 Bass kernel: GatedRecurrentCell (v7, hybrid).

Math (per batch b, channel i, time t):
    pa = x @ Wa^T + ba ; pi = x @ Wi^T + bi
    a  = sigmoid(gate) * 3**(-sigmoid(pa))
    c  = sqrt(1-a^2) * silu(pi + bi)
    h_t = a_t*h_{t-1} + c_t   (h_{-1} = 0);  out = h

Tricks:
 1. 3**(-sigmoid(p)) == FA - FB*tanh(FC*p + FD) to 5.5e-4 abs (global
    least-squares fit), so a = aA + nB*tanh(FC*pa + tb) with per-channel
    aA = sigmoid(gate)*FA, nB = -sigmoid(gate)*FB. Removes the Exp ACT
    pass and its table set; the decay becomes a tensor_scalar affine.
 2. Channels are sorted by alpha = sigmoid(gate) on the host. For the
    low-alpha chunks (alpha < 0.97), sqrt(1-a^2) itself is fit as
    E + F*tanh(G*p + H) (per-channel params interpolated from a table,
    max abs err ~6e-3 at alpha ~0.97, much less below), so those chunks
    need NO a^2 / NO Sqrt: a second Tanh (same table set) + one fused
    scalar_tensor_tensor gives c/F, and the host multiplies those output
    channels by F (the recurrence is linear in c). The remaining chunks
    use the exact a^2 + Sqrt path. The output is un-permuted on the host.

Mapping: data-parallel over batch (8 cores, 1 batch each); channels on
partitions (16 chunks of 128), time on the free dim. GEMMs in bf16.
The recurrence runs as fp32 tensor_tensor_scan on DVE over PAIRS of
chunks concatenated on the free dim (a[pair-start] is zeroed, which
restarts the recurrence). h is written back bf16, upcast on the host.
"""

import functools
import os

import numpy as np

B, S, D, I = 8, 2048, 512, 2048
P = 128
NCORES = 8

# fit of 3^(-sigmoid(p)) = FA - FB*tanh(FC*p + FD), max abs err 5.5e-4
FA = 0.66661083
FB = 0.33324857
FC = 0.5096609
FD = 0.27426951

# alpha threshold for the q-fit (tanh) path
ATHRESH = float(os.environ.get("GRC_ATHRESH", "0.99"))

# knobs: how many of the 16 chunks run each flexible op on GPSIMD
AFF_GP_N = int(os.environ.get("GRC_AFF_GP", "12"))   # a = nB*th + aA
A2_GP_N = int(os.environ.get("GRC_A2_GP", "16"))     # a2 = a*a
A2_ACT_N = int(os.environ.get("GRC_A2_ACT", "16"))    # a2 on ACT (Square)
CMUL_GP_N = int(os.environ.get("GRC_CMUL_GP", "0"))  # c = q*w
CW = int(os.environ.get("GRC_CW", "1024"))           # PSUM supertile width

# per-alpha params of sqrt(1 - alpha^2 (3^-sigmoid(p))^2) ~ E + F*tanh(G*p+H)
QFIT_ALPHAS = [0.88000000, 0.88199333, 0.88398667, 0.88598000, 0.88797333, 0.88996667, 0.89196000, 0.89395333, 0.89594667, 0.89794000, 0.89993333, 0.90192667, 0.90392000, 0.90591333, 0.90790667, 0.90990000, 0.91189333, 0.91388667, 0.91588000, 0.91787333, 0.91986667, 0.92186000, 0.92385333, 0.92584667, 0.92784000, 0.92983333, 0.93182667, 0.93382000, 0.93581333, 0.93780667, 0.93980000, 0.94179333, 0.94378667, 0.94578000, 0.94777333, 0.94976667, 0.95176000, 0.95375333, 0.95574667, 0.95774000, 0.95973333, 0.96172667, 0.96372000, 0.96571333, 0.96770667, 0.96970000, 0.97169333, 0.97368667, 0.97568000, 0.97767333, 0.97966667, 0.98166000, 0.98365333, 0.98564667, 0.98764000, 0.98963333, 0.99162667, 0.99362000, 0.99561333, 0.99760667, 0.99960000]
QFIT_E = [0.71793278, 0.71600446, 0.71405776, 0.71209227, 0.71010754, 0.70810315, 0.70607860, 0.70403341, 0.70196707, 0.69987905, 0.69776877, 0.69563567, 0.69347911, 0.69129846, 0.68909305, 0.68686215, 0.68460502, 0.68232086, 0.68000886, 0.67766812, 0.67529771, 0.67289666, 0.67046391, 0.66799836, 0.66549882, 0.66296404, 0.66039266, 0.65778326, 0.65513429, 0.65244409, 0.64971089, 0.64693275, 0.64410759, 0.64123318, 0.63830704, 0.63532651, 0.63228864, 0.62919024, 0.62602773, 0.62279720, 0.61949430, 0.61611411, 0.61265117, 0.60909930, 0.60545142, 0.60169947, 0.59783411, 0.59384442, 0.58971748, 0.58543774, 0.58098606, 0.57633789, 0.57145993, 0.56630205, 0.56077545, 0.55468643, 0.54756396, 0.53875898, 0.52844085, 0.51626429, 0.50175457]
QFIT_F = [0.24004662, 0.24178957, 0.24355142, 0.24533264, 0.24713369, 0.24895508, 0.25079732, 0.25266095, 0.25454653, 0.25645466, 0.25838594, 0.26034102, 0.26232057, 0.26432530, 0.26635595, 0.26841331, 0.27049820, 0.27261148, 0.27475408, 0.27692696, 0.27913115, 0.28136775, 0.28363790, 0.28594285, 0.28828390, 0.29066246, 0.29308004, 0.29553823, 0.29803879, 0.30058357, 0.30317457, 0.30581400, 0.30850420, 0.31124776, 0.31404750, 0.31690649, 0.31982812, 0.32281615, 0.32587470, 0.32900841, 0.33222243, 0.33552257, 0.33891542, 0.34240846, 0.34601028, 0.34973087, 0.35358188, 0.35757715, 0.36173337, 0.36607116, 0.37061670, 0.37540500, 0.38048620, 0.38594071, 0.39191979, 0.39876686, 0.40733515, 0.41890298, 0.43070055, 0.44102732, 0.45498155]
QFIT_G = [0.51971583, 0.51947071, 0.51921623, 0.51895217, 0.51867828, 0.51839428, 0.51809993, 0.51779490, 0.51747895, 0.51715177, 0.51681308, 0.51646250, 0.51609970, 0.51572428, 0.51533589, 0.51493410, 0.51451843, 0.51408842, 0.51364352, 0.51318321, 0.51270682, 0.51221373, 0.51170317, 0.51117438, 0.51062647, 0.51005850, 0.50946944, 0.50885813, 0.50822330, 0.50756357, 0.50687742, 0.50616313, 0.50541876, 0.50464227, 0.50383124, 0.50298302, 0.50209466, 0.50116277, 0.50018355, 0.49915268, 0.49806518, 0.49691537, 0.49569666, 0.49440143, 0.49302066, 0.49154368, 0.48995774, 0.48824726, 0.48639286, 0.48436981, 0.48214522, 0.47967263, 0.47688059, 0.47364484, 0.46971266, 0.46447851, 0.45642719, 0.44339164, 0.43048843, 0.42141485, 0.40836556]
QFIT_H = [0.67993499, 0.68125400, 0.68258569, 0.68393039, 0.68528845, 0.68666017, 0.68804598, 0.68944616, 0.69086122, 0.69229152, 0.69373760, 0.69519984, 0.69667875, 0.69817483, 0.69968865, 0.70122076, 0.70277170, 0.70434215, 0.70593260, 0.70754385, 0.70917645, 0.71083113, 0.71250863, 0.71420964, 0.71593490, 0.71768522, 0.71946140, 0.72126423, 0.72309453, 0.72495316, 0.72684101, 0.72875892, 0.73070773, 0.73268837, 0.73470169, 0.73674848, 0.73882959, 0.74094574, 0.74309769, 0.74528601, 0.74751107, 0.74977327, 0.75207252, 0.75440855, 0.75678055, 0.75918703, 0.76162556, 0.76409232, 0.76658142, 0.76908360, 0.77158412, 0.77405820, 0.77646132, 0.77870514, 0.78059332, 0.78163255, 0.78056707, 0.77549166, 0.77558611, 0.78883748, 0.79784109]


def _build_nc(s, d, i, nfit=0, silu=True):
    import concourse.bacc as bacc
    import concourse.mybir as mybir
    import concourse.tile as tile
    from concourse.tile import add_dep_helper
    from contextlib import ExitStack

    F32 = mybir.dt.float32
    BF16 = mybir.dt.bfloat16
    AF = mybir.ActivationFunctionType
    ALU = mybir.AluOpType

    nd = d // P            # contraction chunks
    ni = i // P            # channel chunks
    cw = min(CW, s)
    nh = s // cw           # supertiles per channel row
    nmm = cw // 512        # matmuls (N=512) per supertile

    def _gp_set(n):
        if n <= 0:
            return set()
        n = min(n, ni)
        return {int(round(j * ni / n)) % ni for j in range(n)}

    aff_gp = _gp_set(AFF_GP_N)
    a2_gp = _gp_set(A2_GP_N)
    cmul_gp = _gp_set(CMUL_GP_N)

    def pair_groups(ics, singles_at_end=2):
        """Consecutive pairs; optionally keep the last chunks single."""
        ics = list(ics)
        nsing = singles_at_end if len(ics) >= 4 else len(ics) % 2
        body = ics[:len(ics) - nsing] if nsing else ics
        gs = [body[j:j + 2] for j in range(0, len(body), 2)]
        gs += [[ic] for ic in ics[len(ics) - nsing:]] if nsing else []
        return gs

    fit_groups = pair_groups(range(nfit), singles_at_end=0)
    ex_groups = pair_groups(range(nfit, ni), singles_at_end=2)

    nc = bacc.Bacc("TRN2", target_bir_lowering=False, debug=False,
                   num_devices=NCORES)

    xT_d = nc.dram_tensor("xT", [d, s], BF16, kind="ExternalInput").ap()
    waT_d = nc.dram_tensor("WaT", [ni, P, d], BF16, kind="ExternalInput").ap()
    wiT_d = nc.dram_tensor("WiT", [ni, P, d], BF16, kind="ExternalInput").ap()
    aA_d = nc.dram_tensor("aA", [P, ni], F32, kind="ExternalInput").ap()
    nB_d = nc.dram_tensor("nB", [P, ni], F32, kind="ExternalInput").ap()
    tb_d = nc.dram_tensor("tb", [P, ni], F32, kind="ExternalInput").ap()
    sb_d = nc.dram_tensor("sb", [P, ni], F32, kind="ExternalInput").ap()
    qs_d = nc.dram_tensor("qs", [P, ni], F32, kind="ExternalInput").ap()
    qb_d = nc.dram_tensor("qb", [P, ni], F32, kind="ExternalInput").ap()
    qE_d = nc.dram_tensor("qE", [P, ni], F32, kind="ExternalInput").ap()
    qF_d = nc.dram_tensor("qF", [P, ni], F32, kind="ExternalInput").ap()
    out_d = nc.dram_tensor("out", [i, s], BF16, kind="ExternalOutput").ap()

    with tile.TileContext(nc) as tc:
        with ExitStack() as ctx:
            const_pool = ctx.enter_context(tc.tile_pool(name="const", bufs=1))
            xt_pool = ctx.enter_context(tc.tile_pool(name="xt", bufs=1))
            wst_pool = ctx.enter_context(tc.tile_pool(name="wst", bufs=1))
            ps_pool = ctx.enter_context(
                tc.tile_pool(name="mmpsum", bufs=1, space="PSUM"))
            rows = ctx.enter_context(tc.tile_pool(name="rows", bufs=1))

            consts = {}
            for nm, dref in [("aA", aA_d), ("nB", nB_d), ("tb", tb_d),
                             ("sb", sb_d), ("qs", qs_d), ("qb", qb_d),
                             ("qE", qE_d), ("qF", qF_d)]:
                t_ = const_pool.tile([P, ni], F32, name=f"{nm}_t")
                nc.sync.dma_start(t_[:], dref[:])
                consts[nm] = t_

            def cc(nm, ic):
                return consts[nm][:, ic:ic + 1]

            # weight stream tiles; first chunks issued before the bulk x
            w_sbs = {}

            def load_weights(ic):
                wi_sb = wst_pool.tile([P, d], BF16, name=f"wi{ic}", tag="wi",
                                      bufs=3)
                nc.sync.dma_start(wi_sb[:], wiT_d[ic])
                wa_sb = wst_pool.tile([P, d], BF16, name=f"wa{ic}", tag="wa",
                                      bufs=3)
                nc.sync.dma_start(wa_sb[:], waT_d[ic])
                w_sbs[ic] = (wi_sb, wa_sb)

            xT_sb = [xt_pool.tile([P, s], BF16, name=f"xT{k}") for k in
                     range(nd)]
            load_weights(0)
            for k in range(nd):
                nc.sync.dma_start(xT_sb[k][:, 0:cw], xT_d[k * P:(k + 1) * P,
                                                          0:cw])
            load_weights(1)
            for h in range(1, nh):
                for k in range(nd):
                    nc.sync.dma_start(
                        xT_sb[k][:, h * cw:(h + 1) * cw],
                        xT_d[k * P:(k + 1) * P, h * cw:(h + 1) * cw])

            act_chain = []

            def act(out_ap, in_ap, func, **kw):
                inst = nc.scalar.activation(out_ap, in_ap, func, **kw)
                if act_chain:
                    add_dep_helper(inst.ins, act_chain[-1].ins, False,
                                   "act table phase order")
                act_chain.append(inst)
                return inst

            def gemm(ps, w_sb, h):
                for k in range(nd):
                    for m in range(nmm):
                        lo = h * cw + m * 512
                        nc.tensor.matmul(
                            ps[:, m * 512:(m + 1) * 512],
                            w_sb[:, k * P:(k + 1) * P],
                            xT_sb[k][:, lo:lo + 512],
                            start=(k == 0), stop=(k == nd - 1))

            def alloc_pair(g, tag_p, tag_s, dtype, bufs_p, bufs_s):
                if len(g) == 2:
                    return rows.tile([P, 2 * s], dtype, name=f"{tag_p}{g[0]}",
                                     tag=tag_p, bufs=bufs_p)
                return rows.tile([P, s], dtype, name=f"{tag_s}{g[0]}",
                                 tag=tag_s, bufs=bufs_s)

            def scan_and_store(g, ap_t, cp_t):
                w2 = len(g) * s
                tag = "hp" if len(g) == 2 else "hs"
                h_t = rows.tile([P, w2], mybir.dt.bfloat16, name=f"h{g[0]}",
                                tag=tag, bufs=2 if len(g) == 2 else 1)
                nc.vector.tensor_tensor_scan(
                    h_t[:], ap_t[:], cp_t[:], 0.0,
                    op0=ALU.mult, op1=ALU.add)
                for j, ic in enumerate(g):
                    nc.sync.dma_start(out_d[ic * P:(ic + 1) * P, :],
                                      h_t[:, j * s:(j + 1) * s])

            def chunk_front(ic, ap_t, jslot, extra_tanh=None):
                """GEMMs + silu->w + tanh_a->th + affine a into ap slice."""
                if ic not in w_sbs:
                    load_weights(ic)
                wi_sb, wa_sb = w_sbs.pop(ic)
                w_t = rows.tile([P, s], BF16, name=f"w{ic}", tag="w", bufs=6)
                th_t = rows.tile([P, s], F32, name=f"th{ic}", tag="th",
                                 bufs=3)
                for h in range(nh):
                    sl = slice(h * cw, (h + 1) * cw)
                    pi_ps = ps_pool.tile([P, cw], F32, name=f"pi{ic}_{h}",
                                         tag="pi", bufs=max(2, 2048 // cw))
                    gemm(pi_ps, wi_sb, h)
                    if silu:
                        act(w_t[:, sl], pi_ps[:], AF.Silu, bias=cc("sb", ic))
                    else:
                        sg = rows.tile([P, cw], F32, name=f"sg{ic}_{h}",
                                       tag="sg", bufs=3)
                        act(sg[:], pi_ps[:], AF.Sigmoid, bias=cc("sb", ic))
                        pib = rows.tile([P, cw], F32, name=f"pib{ic}_{h}",
                                        tag="pib", bufs=3)
                        act(pib[:], pi_ps[:], AF.Identity, bias=cc("sb", ic))
                        nc.vector.tensor_mul(w_t[:, sl], sg[:], pib[:])
                tq_t = None
                if extra_tanh:
                    tq_t = rows.tile([P, s], BF16, name=f"tq{ic}", tag="tq",
                                     bufs=3)
                for h in range(nh):
                    sl = slice(h * cw, (h + 1) * cw)
                    pa_ps = ps_pool.tile([P, cw], F32, name=f"pa{ic}_{h}",
                                         tag="pa", bufs=max(2, 2048 // cw))
                    gemm(pa_ps, wa_sb, h)
                    act(th_t[:, sl], pa_ps[:], AF.Tanh,
                        scale=FC, bias=cc("tb", ic))
                    if extra_tanh:
                        # q = E + F*tanh(qs*pa + qb); tq holds the tanh
                        act(tq_t[:, sl], pa_ps[:], AF.Tanh,
                            scale=cc("qs", ic), bias=cc("qb", ic))
                a_v = ap_t[:, jslot * s:(jslot + 1) * s]
                aff_eng = nc.gpsimd if ic in aff_gp else nc.vector
                aff_eng.tensor_scalar(
                    a_v, th_t[:], cc("nB", ic), cc("aA", ic),
                    op0=ALU.mult, op1=ALU.add)
                state_th[ic] = th_t
                return w_t, a_v, tq_t

            state_th = {}
            a2_act = set(range(nfit, min(ni, nfit + A2_ACT_N)))

            # ---- fit chunks: one silu-set stream, no a2 / no sqrt --------
            for g in fit_groups:
                ap_t = alloc_pair(g, "ap", "as", F32, 2, 1)
                cp_t = alloc_pair(g, "cp", "cs", BF16, 2, 1)
                for j, ic in enumerate(g):
                    w_t, a_v, tq_t = chunk_front(ic, ap_t, j,
                                                 extra_tanh=True)
                    # q = qF*tq + qE (GPSIMD), then c = q*w (DVE, bf16 2x)
                    q_t = rows.tile([P, s], BF16, name=f"q{ic}", tag="q",
                                    bufs=3)
                    nc.gpsimd.tensor_scalar(
                        q_t[:], tq_t[:], cc("qF", ic), cc("qE", ic),
                        op0=ALU.mult, op1=ALU.add)
                    nc.vector.tensor_mul(cp_t[:, j * s:(j + 1) * s], q_t[:],
                                         w_t[:])
                    if j == 1:
                        nc.gpsimd.memset(ap_t[:, s:s + 1], 0.0)
                scan_and_store(g, ap_t, cp_t)

            # ---- exact chunks: lag-1 pair-group pipeline ----------------
            state = {}

            def phase_ex_a(g):
                ap_t = alloc_pair(g, "ap", "as", F32, 2, 1)
                st = {"apair": ap_t, "g": list(g), "w": {}, "a2": {}}
                for j, ic in enumerate(g):
                    w_t, a_v, _ = chunk_front(ic, ap_t, j)
                    a2_t = rows.tile([P, s], F32, name=f"a2{ic}", tag="a2",
                                     bufs=4)
                    if ic in a2_act:
                        # a2 = (nB*th + aA)^2 on ACT; Square is in every
                        # table set so this costs no table switch
                        act(a2_t[:], state_th[ic][:], AF.Square,
                            scale=cc("nB", ic), bias=cc("aA", ic))
                    else:
                        a2_eng = nc.gpsimd if ic in a2_gp else nc.vector
                        a2_eng.tensor_mul(a2_t[:], a_v, a_v)
                    st["w"][ic] = w_t
                    st["a2"][ic] = a2_t
                    if j == 1:
                        nc.gpsimd.memset(ap_t[:, s:s + 1], 0.0)
                state[g[0]] = st

            def phase_ex_b(g):
                st = state.pop(g[0])
                ap_t = st["apair"]
                cp_t = alloc_pair(g, "cp", "cs", BF16, 2, 1)
                for j, ic in enumerate(g):
                    q_t = rows.tile([P, s], BF16, name=f"q{ic}", tag="q",
                                    bufs=3)
                    act(q_t[:], st["a2"][ic][:], AF.Sqrt,
                        scale=-1.0, bias=1.0)
                    cm_eng = nc.gpsimd if ic in cmul_gp else nc.vector
                    cm_eng.tensor_mul(cp_t[:, j * s:(j + 1) * s], q_t[:],
                                      st["w"][ic][:])
                scan_and_store(g, ap_t, cp_t)

            prev = None
            for g in ex_groups:
                phase_ex_a(g)
                if prev is not None:
                    phase_ex_b(prev)
                prev = g
            if prev is not None:
                phase_ex_b(prev)

    nc.compile()
    return nc


@functools.lru_cache(maxsize=4)
def _get_nc(s=S, d=D, i=I, nfit=0):
    return _build_nc(s, d, i, nfit=nfit)


LAST_RESULTS = None


def _prep_core_inputs(xb, shared):
    import ml_dtypes
    xT = np.ascontiguousarray(xb.T).astype(ml_dtypes.bfloat16)
    m = {"xT": xT}
    m.update(shared)
    return m


def _prep_shared(Wa, ba, Wi, bi, gate, d, i):
    """Sort channels by alpha, build device inputs. Returns
    (shared dict, nfit, perm, out_scale[i])."""
    import ml_dtypes
    ni = i // P
    nd = d // P
    alpha_u = 1.0 / (1.0 + np.exp(-gate.astype(np.float64)))
    perm = np.argsort(alpha_u, kind="stable")
    Wa = Wa[perm]
    Wi = Wi[perm]
    ba = ba[perm]
    bi = bi[perm]
    alpha = alpha_u[perm]

    nfit = int((alpha < ATHRESH).sum()) // P
    nfit = min(nfit, ni)

    WaT = np.ascontiguousarray(
        Wa.reshape(ni, P, nd, P).transpose(0, 3, 2, 1).reshape(ni, P, d)
    ).astype(ml_dtypes.bfloat16)
    WiT = np.ascontiguousarray(
        Wi.reshape(ni, P, nd, P).transpose(0, 3, 2, 1).reshape(ni, P, d)
    ).astype(ml_dtypes.bfloat16)

    aA = (alpha * FA).astype(np.float32)
    nB = (-alpha * FB).astype(np.float32)
    tb = (FC * ba.astype(np.float64) + FD).astype(np.float32)
    sb = bi.astype(np.float32)

    qs = np.zeros(i, np.float32)
    qb = np.zeros(i, np.float32)
    qE = np.zeros(i, np.float32)
    qF = np.zeros(i, np.float32)
    scale = np.ones(i, np.float64)
    if nfit > 0:
        nf = nfit * P
        al = np.clip(alpha[:nf], QFIT_ALPHAS[0], QFIT_ALPHAS[-1])
        E = np.interp(al, QFIT_ALPHAS, QFIT_E)
        F = np.interp(al, QFIT_ALPHAS, QFIT_F)
        G = np.interp(al, QFIT_ALPHAS, QFIT_G)
        H = np.interp(al, QFIT_ALPHAS, QFIT_H)
        qs[:nf] = G
        qb[:nf] = G * ba.astype(np.float64)[:nf] + H
        qE[:nf] = E
        qF[:nf] = F

    def vec(v):
        return np.ascontiguousarray(v.astype(np.float32).reshape(ni, P).T)

    shared = {"WaT": WaT, "WiT": WiT, "aA": vec(aA), "nB": vec(nB),
              "tb": vec(tb), "sb": vec(sb), "qs": vec(qs), "qb": vec(qb),
              "qE": vec(qE), "qF": vec(qF)}
    return shared, nfit, perm, scale.astype(np.float32)


def kernel(x, Wa, ba, Wi, bi, gate):
    global LAST_RESULTS
    from concourse.bass_utils import run_bass_kernel_spmd

    x = np.asarray(x, dtype=np.float32)
    b, s, d = x.shape
    i = Wa.shape[0]

    shared, nfit, perm, oscale = _prep_shared(
        np.asarray(Wa, np.float32), np.asarray(ba, np.float32),
        np.asarray(Wi, np.float32), np.asarray(bi, np.float32),
        np.asarray(gate, np.float32), d, i)
    nc = _get_nc(s, d, i, nfit)

    in_maps = [_prep_core_inputs(x[bb], shared) for bb in range(b)]
    res = run_bass_kernel_spmd(nc, in_maps, list(range(b)))
    LAST_RESULTS = res
    out = np.empty((b, s, i), np.float32)
    for bb in range(b):
        hs = np.asarray(res.results[bb]["out"]).astype(np.float32).T * oscale
        out[bb, :, perm] = hs.T
    return out

